# revision 1
# baseline (speedup 1.0000x reference)
"""3-layer stacked LSTM regressor (B=128, T=256, IN=64, H=512) on 8 TRN2
NeuronCores.

Strategy: layer-wavefront pipeline. Cores 0,1,2 own LSTM layers 0,1,2.
Time is split into C chunks of S steps. In "slot" s, core r runs chunk
s-r of its layer; an 8-rank AllGather at the end of each slot ships
every core's h^T chunk sequence, and core c consumes section (c-1)%8
the next slot. Roles are encoded purely in per-core DATA (weights and
x-chunk content), so the SPMD program is identical on all cores.

Matmuls keep the gates transposed (G^T tiles [128 units, 128 batch],
weights stationary fp16) so the hidden state never needs a transpose.
The bias enters through a ones-row (row 64) of the x-chunk rhs, which
is only set for real chunks — garbage pipeline-fill chunks therefore
keep the state at exactly zero. c-state stays fp32.

The final h2(T) tile is pulled from rank 2; the tiny [128,512]@[512,1]
output projection and MSE loss run on host.
"""

import sys

sys.path.insert(0, "/opt/trn_rl_repo")

import numpy as np

from concourse import bass, bacc, tile, mybir
from concourse import bass_utils

F16 = mybir.dt.float16
F32 = mybir.dt.float32

B, T, IN, H, L, OUT = 128, 256, 64, 512, 3, 1
G4 = 4 * H
NCORES = 8
M_TILES = G4 // 128
KH = H // 128
NK_IH = 1 + KH
S, C = 16, 16  # steps per chunk, chunks


def _build_nc(S, C, num_devices=NCORES):
    SLOTS = C + 2
    nc = bacc.Bacc(
        "TRN2",
        target_bir_lowering=False,
        debug=False,
        enable_asserts=False,
        num_devices=num_devices,
    )

    xck = nc.dram_tensor("xck", [SLOTS, S, 128, 128], F16, kind="ExternalInput")
    w_ih = nc.dram_tensor("w_ih", [NK_IH, 128, G4], F16, kind="ExternalInput")
    w_hh = nc.dram_tensor("w_hh", [KH, 128, G4], F16, kind="ExternalInput")
    ht_out = nc.dram_tensor("ht_out", [128, H], F32, kind="ExternalOutput")

    contrib_dram = [
        nc.dram_tensor(f"contrib_dram{i}", [128, S * H], F16) for i in range(2)
    ]
    agout_dram = [
        nc.dram_tensor(
            f"agout_dram{i}", [NCORES * 128, S * H], F16, addr_space="Shared"
        )
        for i in range(2)
    ]

    xck_ap = xck.ap().rearrange("s t p b -> s p t b")

    with tile.TileContext(nc) as tc:
        with (
            tc.tile_pool(name="wpool", bufs=1) as wpool,
            tc.tile_pool(name="xpool", bufs=2) as xpool,
            tc.tile_pool(name="hpool", bufs=2) as hpool,
            tc.tile_pool(name="cpool", bufs=3) as cpool,
            tc.tile_pool(name="gpool", bufs=2) as gpool,
            tc.tile_pool(name="spool", bufs=1) as spool,
            tc.tile_pool(name="ppool", bufs=2, space="PSUM") as ppool,
        ):
            w_ih_sb = wpool.tile([128, NK_IH * G4], F16, tag="wih")
            w_hh_sb = wpool.tile([128, KH * G4], F16, tag="whh")
            nc.sync.dma_start(
                out=w_ih_sb[:].rearrange("p (k c) -> p k c", k=NK_IH),
                in_=w_ih.ap().rearrange("k p c -> p k c"),
            )
            nc.sync.dma_start(
                out=w_hh_sb[:].rearrange("p (k c) -> p k c", k=KH),
                in_=w_hh.ap().rearrange("k p c -> p k c"),
            )
            cT = spool.tile([128, H], F32, tag="cT")
            nc.vector.memset(cT[:], 0.0)

            prev_contrib = cpool.tile([128, S * H], F16, tag="contrib")
            nc.vector.memset(prev_contrib[:], 0.0)

            pid = nc.gpsimd.partition_id()
            row_reg = ((pid + (NCORES - 1)) % NCORES) * 128

            def wih_tile(k, m):
                return w_ih_sb[:, k * G4 + m * 128 : k * G4 + (m + 1) * 128]

            def whh_tile(k, m):
                return w_hh_sb[:, k * G4 + m * 128 : k * G4 + (m + 1) * 128]

            Sig = mybir.ActivationFunctionType.Sigmoid
            Tanh = mybir.ActivationFunctionType.Tanh

            for s in range(SLOTS):
                inp_x = xpool.tile([128, S * 128], F16, tag="inp_x")
                nc.sync.dma_start(
                    out=inp_x[:].rearrange("p (t b) -> p t b", t=S),
                    in_=xck_ap[s],
                )
                inp_h = hpool.tile([128, S * H], F16, tag="inp_h")
                if s == 0:
                    nc.vector.memset(inp_h[:], 0.0)
                else:
                    nc.gpsimd.dma_start(
                        out=inp_h[:],
                        in_=agout_dram[(s - 1) % 2].ap()[bass.ds(row_reg, 128), :],
                    )
                contrib = cpool.tile([128, S * H], F16, tag="contrib")

                for t in range(S):
                    psum = ppool.tile([128, G4], F32, tag="psum")
                    h_prev = (
                        contrib[:, (t - 1) * H : t * H]
                        if t > 0
                        else prev_contrib[:, (S - 1) * H : S * H]
                    )
                    for m in range(M_TILES):
                        po = psum[:, m * 128 : (m + 1) * 128]
                        nc.tensor.matmul(
                            po,
                            wih_tile(0, m),
                            inp_x[:, t * 128 : (t + 1) * 128],
                            start=True,
                            stop=False,
                        )
                        for k in range(KH):
                            nc.tensor.matmul(
                                po,
                                wih_tile(1 + k, m),
                                inp_h[:, t * H + k * 128 : t * H + (k + 1) * 128],
                                start=False,
                                stop=False,
                            )
                        for k in range(KH):
                            nc.tensor.matmul(
                                po,
                                whh_tile(k, m),
                                h_prev[:, k * 128 : (k + 1) * 128],
                                start=False,
                                stop=(k == KH - 1),
                            )
                    si = gpool.tile([128, H], F32, tag="si")
                    sf = gpool.tile([128, H], F32, tag="sf")
                    tg = gpool.tile([128, H], F32, tag="tg")
                    so = gpool.tile([128, H], F32, tag="so")
                    nc.scalar.activation(si[:], psum[:, 0:H], Sig)
                    nc.scalar.activation(sf[:], psum[:, H : 2 * H], Sig)
                    nc.scalar.activation(tg[:], psum[:, 2 * H : 3 * H], Tanh)
                    nc.scalar.activation(so[:], psum[:, 3 * H : 4 * H], Sig)
                    t1 = gpool.tile([128, H], F32, tag="t1")
                    t2 = gpool.tile([128, H], F32, tag="t2")
                    nc.vector.tensor_mul(t1[:], sf[:], cT[:])
                    nc.vector.tensor_mul(t2[:], si[:], tg[:])
                    nc.vector.tensor_add(cT[:], t1[:], t2[:])
                    tc2 = gpool.tile([128, H], F32, tag="tc2")
                    nc.scalar.activation(tc2[:], cT[:], Tanh)
                    nc.vector.tensor_mul(
                        contrib[:, t * H : (t + 1) * H], so[:], tc2[:]
                    )

                bounce = contrib_dram[s % 2]
                nc.sync.dma_start(out=bounce.ap(), in_=contrib[:])
                nc.gpsimd.collective_compute(
                    "AllGather",
                    mybir.AluOpType.bypass,
                    replica_groups=[list(range(NCORES))],
                    ins=[bounce.ap().opt()],
                    outs=[agout_dram[s % 2].ap().opt()],
                )
                prev_contrib = contrib

            fin = gpool.tile([128, H], F32, tag="fin")
            nc.vector.tensor_copy(fin[:], prev_contrib[:, (S - 1) * H : S * H])
            nc.sync.dma_start(out=ht_out.ap(), in_=fin[:])

    nc.compile()
    return nc


def _build_in_maps(x, Ws, S, C):
    SLOTS = C + 2
    in_maps = []
    for core in range(NCORES):
        xck = np.zeros((SLOTS, S, 128, 128), np.float32)
        wih = np.zeros((NK_IH, 128, G4), np.float32)
        whh = np.zeros((KH, 128, G4), np.float32)
        role = core if core < L else None
        if role is not None:
            Wih, Whh, bih, bhh = Ws[role]
            bias = (bih + bhh).astype(np.float32)
            if role == 0:
                for s in range(C):
                    for t in range(S):
                        xck[s, t, :IN, :] = x[:, s * S + t, :].T
                    xck[s, :, IN, :] = 1.0
                wih[0, :IN, :] = Wih.T
                wih[0, IN, :] = bias
            else:
                for s in range(SLOTS):
                    if role <= s < C + role:
                        xck[s, :, IN, :] = 1.0
                wih[0, IN, :] = bias
                for k in range(KH):
                    wih[1 + k] = Wih.T[k * 128 : (k + 1) * 128, :]
            for k in range(KH):
                whh[k] = Whh.T[k * 128 : (k + 1) * 128, :]
        in_maps.append(
            {
                "xck": xck.astype(np.float16),
                "w_ih": wih.astype(np.float16),
                "w_hh": whh.astype(np.float16),
            }
        )
    return in_maps


_nc_cache = {}


def kernel(x, y, W_ih0, W_hh0, b_ih0, b_hh0, W_ih_r, W_hh_r, b_ih_r, b_hh_r,
           W_out, b_out):
    x = np.asarray(x, np.float32)
    y = np.asarray(y, np.float32)
    Ws = [(np.asarray(W_ih0), np.asarray(W_hh0),
           np.asarray(b_ih0), np.asarray(b_hh0))]
    for l in range(L - 1):
        Ws.append((np.asarray(W_ih_r[l]), np.asarray(W_hh_r[l]),
                   np.asarray(b_ih_r[l]), np.asarray(b_hh_r[l])))

    key = (S, C)
    if key not in _nc_cache:
        _nc_cache[key] = _build_nc(S, C)
    nc = _nc_cache[key]
    in_maps = _build_in_maps(x, Ws, S, C)
    res = bass_utils.run_bass_kernel_spmd(
        nc, in_maps, core_ids=list(range(NCORES))
    )
    ht = res.results[2]["ht_out"]  # rank 2 = layer 2

    h2 = np.zeros((B, H), np.float32)
    for k in range(KH):
        h2[:, k * 128 : (k + 1) * 128] = ht[:, k * 128 : (k + 1) * 128].T

    out = (h2 @ np.asarray(W_out, np.float32).T + np.asarray(b_out, np.float32))
    out = out.astype(np.float32)
    loss = np.float32(np.mean((out - y) ** 2))
    return out, loss
